# revision 58
# baseline (speedup 1.0000x reference)
"""AttentionBlock (GroupNorm + 8-head attention + proj + residual) on 8 TRN2 NeuronCores.

Data-parallel over batch: 16 batches -> 2 per core, no collectives.
Per-core layout (channels on partitions, spatial T=1024 on the free dim):
  - GroupNorm(32, 512): bn_stats per channel, cross-partition group
    aggregation via tiny PE matmuls, normalize as x*A + B per channel.
  - qkv: bf16 matmuls; Q,K as [c, t] with head h on partition half 64*(h%2)
    of j-tile h//2; V pre-transposed [s, c] with a ones column per head so
    softmax denominators fall out of the AV matmul for free. Even-parity
    heads' V goes to fp8e4 (for DoubleRow AV), odd heads' to bf16.
  - attention per head-pair (2i, 2i+1), two t-half phases so AV accumulators
    take one PSUM bank each and QK gets 6 banks of slot depth:
      phase 0: per s-tile, both heads' K=64 QK^T matmuls (row-tiled via
      partitions 0-63/64-127), exp evictions split by head parity across
      ScalarE (ACT Exp -> fp8, logits shifted -2 to fit e4m3) and VectorE
      (Schraudolph fast-exp: bf16 bits = int16(x*184.66+16251)), plus the
      first t-half AV; phase 1: pure AV burst over the second t-half.
      Even-head AV uses fp8 DoubleRow over s-tile pairs (half the matmuls).
  - softmax normalize: accs staged to SBUF bf16 (frees PSUM ~1.2us after the
    last AV); 1/denominator via a bf16 bit-trick seed + 2 Newton steps on
    DVE (no ACT table loads); gpsimd partition-broadcast; one multiply per
    head/t-half. The whole chain is deferred one head-pair so it never
    head-of-line-blocks the engine queues.
  - PE idle windows are filled: warm-up matmuls during the x-load/GN
    prologue (keeps the HAM clock gate at full rate), next-batch qkv m-tiles
    at batch-0 head-pair boundaries, previous-batch proj m-tiles during
    batch-1 attention.
"""

import numpy as np
from contextlib import ExitStack

import concourse.bass as bass
import concourse.tile as tile
from concourse import mybir
from concourse.bass_utils import run_bass_kernel_spmd

B, C, T = 16, 512, 1024
NH, CH = 8, 64
GS = 16  # channels per GroupNorm group
EPS = 1e-5
NCORES = 8
BL = B // NCORES  # batches per core
P = 128
F32 = mybir.dt.float32
BF16 = mybir.dt.bfloat16
I16 = mybir.dt.int16
FP8 = mybir.dt.float8e4
DR = mybir.MatmulPerfMode.DoubleRow
AF = mybir.ActivationFunctionType
OP = mybir.AluOpType

VT_W = 65  # per-head V^T columns: 64 channels + 1 ones column

# Schraudolph fast-exp constants (bf16 bit pattern via int16)
SCH_A = 184.66496543257098  # 128 * log2(e)
SCH_B = 16251.0             # calibrated for truncation toward zero
RCP_MAGIC = 32493.0         # bf16 bit-trick reciprocal seed (int16 domain)


def _kernel_body(nc, tc, ap, out_ap):
    ctx = tc._ctx  # ExitStack attached by build()

    const = ctx.enter_context(tc.tile_pool(name="const", bufs=1))
    gn_pool = ctx.enter_context(tc.tile_pool(name="gn", bufs=2))
    qk_pool = ctx.enter_context(tc.tile_pool(name="qk", bufs=1))
    ew_pool = ctx.enter_context(tc.tile_pool(name="ew", bufs=12))
    ew8_pool = ctx.enter_context(tc.tile_pool(name="ew8", bufs=6))
    rc_pool = ctx.enter_context(tc.tile_pool(name="rc", bufs=1))
    stf_pool = ctx.enter_context(tc.tile_pool(name="stf", bufs=8))
    outp = ctx.enter_context(tc.tile_pool(name="outp", bufs=3))
    psqk = ctx.enter_context(tc.tile_pool(name="psqk", bufs=3, space="PSUM"))
    psacc = ctx.enter_context(tc.tile_pool(name="psacc", bufs=2, space="PSUM"))

    xv = ap["x"].rearrange("b (j p) t -> b j p t", p=P)  # [BL, 4, 128, T]
    ov = out_ap.rearrange("b (j p) t -> b j p t", p=P)

    # ---------------- x load first (GroupNorm needs it immediately) --------
    xf = const.tile([P, BL, 4, T], BF16)  # raw x (bf16 copy for GN/h)
    xh = const.tile([P, BL, 4, T], BF16)  # normalized h (bf16)
    xvr = ap["xbf"].rearrange("b (j p) t -> b j p t", p=P)
    _dmaq = [nc.sync, nc.scalar, nc.gpsimd, nc.sync]
    for b in range(BL):
        for j in range(4):
            _dmaq[j].dma_start(out=xf[:, b, j, :], in_=xvr[b, j])

    # ---------------- constants ----------------
    wq_sb = const.tile([P, 4, 3 * C], BF16)  # w_qkv^T: [cin_part, cin_tile, out]
    nc.sync.dma_start(out=wq_sb, in_=ap["wqkvT"].rearrange("(j p) o -> p j o", p=P))
    wp_sb = const.tile([P, 4, C], BF16)  # w_proj^T
    nc.sync.dma_start(out=wp_sb, in_=ap["wprojT"].rearrange("(j p) o -> p j o", p=P))
    bqk_sb = const.tile([P, 8], F32)
    nc.sync.dma_start(out=bqk_sb, in_=ap["bqk"])
    bp_sb = const.tile([P, 4], F32)
    nc.sync.dma_start(out=bp_sb, in_=ap["bp"])
    gscale_sb = const.tile([P, 4], F32)
    nc.sync.dma_start(out=gscale_sb, in_=ap["gscale"])
    gbias_sb = const.tile([P, 4], F32)
    nc.sync.dma_start(out=gbias_sb, in_=ap["gbias"])
    gsel_sb = const.tile([P, 8], F32)
    nc.sync.dma_start(out=gsel_sb, in_=ap["gsel"])
    gexp_sb = const.tile([8, P], F32)
    nc.sync.dma_start(out=gexp_sb, in_=ap["gexp"])
    bv_sb = const.tile([P, C], F32)  # V bias broadcast across partitions
    nc.sync.dma_start(out=bv_sb, in_=ap["bv"].partition_broadcast(P))
    eps_sb = const.tile([8, 1], F32)
    nc.vector.memset(eps_sb, EPS)
    expwarm = const.tile([1, 1], F32)
    nc.scalar.activation(out=expwarm, in_=eps_sb[0:1, :], func=AF.Exp)
    nb2_sb = const.tile([P, 1], F32)  # -2.0 bias for fp8-range exp shift
    nc.vector.memset(nb2_sb, -2.0)
    scr_sb = const.tile([P, 512], BF16)  # scratch operand for PE warm-up
    nc.vector.memset(scr_sb, 0.5)
    ones1_sb = const.tile([1, P], BF16)  # K=1 ones row for V-bias matmul fold
    nc.vector.memset(ones1_sb, 1.0)
    bvrow_sb = const.tile([1, C], BF16)
    nc.sync.dma_start(out=bvrow_sb, in_=ap["bvbf"])

    def pe_warmup(n_mm):
        dmy = psacc.tile([VT_W, 512], F32, tag="acc")
        for _ in range(n_mm):
            nc.tensor.matmul(
                out=dmy, lhsT=scr_sb[:, 0:VT_W], rhs=scr_sb, start=True, stop=True
            )

    pe_warmup(40)


    # ---------------- GroupNorm (both batches) ----------------
    for b in range(BL):
        bnraw = gn_pool.tile([P, 4, 2, 6], F32, tag="bnraw")
        mv = gn_pool.tile([P, 4, 2], F32, tag="mv")
        for j in range(4):
            for hf in range(2):
                nc.vector.bn_stats(
                    out=bnraw[:, j, hf, :], in_=xf[:, b, j, 512 * hf : 512 * (hf + 1)]
                )
            nc.vector.bn_aggr(out=mv[:, j, :], in_=bnraw[:, j, :, :])
        # m2: cols 0-3 per-channel mean (per c-tile), cols 4-7 per-channel E[x^2]
        m2 = gn_pool.tile([P, 8], F32, tag="m2")
        nc.vector.tensor_copy(out=m2[:, 0:4], in_=mv[:, :, 0])
        nc.vector.tensor_mul(out=m2[:, 4:8], in0=mv[:, :, 0], in1=mv[:, :, 0])
        nc.vector.tensor_add(out=m2[:, 4:8], in0=m2[:, 4:8], in1=mv[:, :, 1])
        # group-aggregate across the 16-channel groups (partition dim) on PE
        gst_ps = psqk.tile([P, T], F32, tag="qk")
        nc.tensor.matmul(
            out=gst_ps[0:8, 0:8], lhsT=gsel_sb, rhs=m2, start=True, stop=True
        )
        gs = gn_pool.tile([8, 8], F32, tag="gs")  # cols 0-3 mu_g, 4-7 E2_g
        nc.vector.tensor_scalar_mul(out=gs, in0=gst_ps[0:8, 0:8], scalar1=1.0 / GS)
        musq = gn_pool.tile([8, 4], F32, tag="musq")
        nc.vector.tensor_mul(out=musq, in0=gs[:, 0:4], in1=gs[:, 0:4])
        var = gn_pool.tile([8, 4], F32, tag="var")
        nc.vector.tensor_sub(out=var, in0=gs[:, 4:8], in1=musq)
        # rstd = 1/sqrt(var+eps), one Newton-Raphson refinement for accuracy
        sq = gn_pool.tile([8, 4], F32, tag="sq")
        nc.scalar.activation(out=sq, in_=var, func=AF.Sqrt, bias=eps_sb)
        y0 = gn_pool.tile([8, 4], F32, tag="y0")
        nc.vector.reciprocal(out=y0, in_=sq)
        t1 = gn_pool.tile([8, 4], F32, tag="t1")
        nc.vector.tensor_mul(out=t1, in0=y0, in1=y0)
        vpe = gn_pool.tile([8, 4], F32, tag="vpe")
        nc.vector.tensor_scalar_add(out=vpe, in0=var, scalar1=EPS)
        nc.vector.tensor_mul(out=t1, in0=t1, in1=vpe)
        nc.vector.tensor_scalar(
            out=t1, in0=t1, scalar1=-0.5, scalar2=1.5, op0=OP.mult, op1=OP.add
        )
        nc.vector.tensor_mul(out=gs[:, 4:8], in0=y0, in1=t1)  # rstd into gs cols 4-7
        # expand group stats back to per-channel on PE
        pc_ps = psqk.tile([P, T], F32, tag="qk")
        nc.tensor.matmul(
            out=pc_ps[:, 0:8], lhsT=gexp_sb, rhs=gs, start=True, stop=True
        )
        pc = gn_pool.tile([P, 8], F32, tag="pc")
        nc.vector.tensor_copy(out=pc, in_=pc_ps[:, 0:8])
        at = gn_pool.tile([P, 4], F32, tag="at")  # A = rstd*gamma
        nc.vector.tensor_mul(out=at, in0=pc[:, 4:8], in1=gscale_sb)
        bt = gn_pool.tile([P, 4], F32, tag="bt")  # B = beta - mu*A
        nc.vector.tensor_mul(out=bt, in0=pc[:, 0:4], in1=at)
        nc.vector.tensor_sub(out=bt, in0=gbias_sb, in1=bt)
        for j in range(4):
            nc.scalar.activation(
                out=xh[:, b, j, :],
                in_=xf[:, b, j, :],
                func=AF.Identity,
                bias=bt[:, j : j + 1],
                scale=at[:, j : j + 1],
            )

    # ---------------- qkv building blocks ----------------
    q_sb = qk_pool.tile([P, BL, 4, T], BF16, tag="q")
    k_sb = qk_pool.tile([P, BL, 4, T], BF16, tag="k")
    # V^T split by head parity: even heads fp8 (DoubleRow AV over s-pairs,
    # 68-wide slots keep the ko stride 16B-aligned), odd heads bf16.
    vt8_sb = qk_pool.tile([P, BL, 4, 2, 4 * 68], FP8, tag="vt8")
    vtb_sb = qk_pool.tile([P, BL, 8, 4 * VT_W], BF16, tag="vtb")
    a_sb = qk_pool.tile([P, BL, 4, T], BF16, tag="a")

    def qkv_qk_mtile(b, m):
        """Q (m<4) or K (m>=4) m-tile: matmul + ScalarE bias eviction."""
        ps = psqk.tile([P, T], F32, tag="qk")
        for j in range(4):
            for n in range(2):
                nc.tensor.matmul(
                    out=ps[:, 512 * n : 512 * (n + 1)],
                    lhsT=wq_sb[:, j, P * m : P * (m + 1)],
                    rhs=xh[:, b, j, 512 * n : 512 * (n + 1)],
                    start=(j == 0),
                    stop=(j == 3),
                )
        dst = q_sb[:, b, m, :] if m < 4 else k_sb[:, b, m - 4, :]
        nc.scalar.activation(
            out=dst, in_=ps, func=AF.Identity, bias=bqk_sb[:, m : m + 1]
        )

    def qkv_v_stile(b, s):
        """V^T s-tile: matmul (xh stationary) + DVE bias eviction."""
        ps_full = psqk.tile([P, T], F32, tag="qk")
        ps = ps_full[:, 0:C]
        for j in range(4):
            nc.tensor.matmul(
                out=ps,
                lhsT=xh[:, b, j, P * s : P * (s + 1)],
                rhs=wq_sb[:, j, 2 * C : 3 * C],
                start=(j == 0),
                stop=(j == 3),
            )
        psr = ps.rearrange("p (h2 par c) -> p par h2 c", par=2, c=CH)
        bvr = bv_sb.rearrange("p (h2 par c) -> p par h2 c", par=2, c=CH)
        dst8 = vt8_sb[:, b, s // 2, s % 2, :].rearrange(
            "p (i w) -> p i w", w=68
        )[:, :, 0:CH]
        nc.vector.tensor_tensor(out=dst8, in0=psr[:, 0], in1=bvr[:, 0], op=OP.add)
        dstb = vtb_sb[:, b, s, :].rearrange("p (i w) -> p i w", w=VT_W)[:, :, 0:CH]
        nc.vector.tensor_tensor(out=dstb, in0=psr[:, 1], in1=bvr[:, 1], op=OP.add)

    def proj_mtile(b, m, xr_t):
        """proj m-tile: matmul + bias/residual fused eviction + store."""
        pps = psqk.tile([P, T], F32, tag="qk")
        for j in range(4):
            for n in range(2):
                nc.tensor.matmul(
                    out=pps[:, 512 * n : 512 * (n + 1)],
                    lhsT=wp_sb[:, j, P * m : P * (m + 1)],
                    rhs=a_sb[:, b, j, 512 * n : 512 * (n + 1)],
                    start=(j == 0),
                    stop=(j == 3),
                )
        o_t = outp.tile([P, T], F32, tag="o")
        for half in range(2):
            hsl = slice(512 * half, 512 * (half + 1))
            nc.vector.scalar_tensor_tensor(
                out=o_t[:, hsl],
                in0=pps[:, hsl],
                scalar=bp_sb[:, m : m + 1],
                in1=xr_t[:, hsl],
                op0=OP.add,
                op1=OP.add,
            )
            nc.sync.dma_start(out=ov[b, m][:, hsl], in_=o_t[:, hsl])

    # ones columns of V^T (written once; V evictions never touch them)
    for b in range(BL):
        ones8 = vt8_sb[:, b].rearrange("p sp ko (i w) -> p sp ko i w", w=68)[
            :, :, :, :, CH : CH + 1
        ]
        nc.vector.memset(ones8, 1.0)
        for s in range(8):
            ones_view = vtb_sb[:, b, s, :].rearrange("p (i w) -> p i w", w=VT_W)[
                :, :, CH : CH + 1
            ]
            nc.vector.memset(ones_view, 1.0)

    # ---------------- attention head-pair ----------------
    def expev(sps, ew, on_dve):
        # whole-tile exp per engine: ACT Exp on ScalarE, Schraudolph on DVE
        if on_dve:
            nc.vector.tensor_scalar(
                out=ew.bitcast(I16),
                in0=sps,
                scalar1=SCH_A,
                scalar2=SCH_B,
                op0=OP.mult,
                op1=OP.add,
            )
        else:
            nc.scalar.activation(out=ew, in_=sps, func=AF.Exp)

    def attn_headpair(b, hp):
        he, ho = 2 * hp, 2 * hp + 1
        jt = hp
        ew_tiles = {}
        accs = {}
        stfs = {}

        def qk_pair(s):
            for h in (he, ho):
                pof = 64 * (h % 2)
                sps = psqk.tile([P, T], F32, tag="qk")
                for n in range(2):
                    nc.tensor.matmul(
                        out=sps[:, 512 * n : 512 * (n + 1)],
                        lhsT=k_sb[pof : pof + 64, b, jt, P * s : P * (s + 1)],
                        rhs=q_sb[pof : pof + 64, b, jt, 512 * n : 512 * (n + 1)],
                        start=True,
                        stop=True,
                    )
                if h % 2 == 0:
                    if s % 2 == 0:
                        ew8 = ew8_pool.tile([P, 2, T], FP8, tag="ew8")
                        ew_tiles[(h, s // 2)] = ew8
                    # -2 logit shift keeps exp within fp8e4 range (the
                    # ones-column denominator scales identically, so the
                    # softmax ratio is unchanged)
                    nc.scalar.activation(
                        out=ew_tiles[(h, s // 2)][:, s % 2, :],
                        in_=sps,
                        func=AF.Exp,
                        bias=nb2_sb,
                    )
                else:
                    ew = ew_pool.tile([P, T], BF16, tag="ew")
                    expev(sps, ew, True)
                    ew_tiles[(h, s)] = ew

        def av(s, n):
            nsl = slice(512 * n, 512 * (n + 1))
            nc.tensor.matmul(
                out=accs[(ho, n)],
                lhsT=vtb_sb[:, b, s, :].rearrange("p (i w) -> p i w", w=VT_W)[
                    :, ho // 2, :
                ],
                rhs=ew_tiles[(ho, s)][:, nsl],
                start=(s == 0),
                stop=(s == 7),
            )
            if s % 2 == 1:
                nc.tensor.matmul(
                    out=accs[(he, n)],
                    lhsT=vt8_sb[:, b, s // 2, :, :].rearrange(
                        "p ko (i w) -> p ko i w", w=68
                    )[:, :, he // 2, 0:VT_W],
                    rhs=ew_tiles[(he, s // 2)][:, :, nsl],
                    start=(s == 1),
                    stop=(s == 7),
                    perf_mode=DR,
                )

        def stage(h, n):
            stf = stf_pool.tile([VT_W, 512], BF16, tag="stf")
            nc.scalar.copy(out=stf, in_=accs[(h, n)])
            stfs[(h, n)] = stf

        # phase 0: all QK+exp, AV over the first t-half (accs 1 bank/head)
        for h in (he, ho):
            acc_t = psacc.tile([VT_W, 512], F32, tag="acc")
            accs[(h, 0)] = acc_t
        for s in range(8):
            qk_pair(s)
            if s >= 1:
                av(s - 1, 0)
        av(7, 0)
        for h in (he, ho):
            stage(h, 0)
        # phase 1: pure AV burst over the second t-half (no exp gating)
        for h in (he, ho):
            acc_t = psacc.tile([VT_W, 512], F32, tag="acc")
            accs[(h, 1)] = acc_t
        for s in range(8):
            av(s, 1)
        for h in (he, ho):
            stage(h, 1)

        def normalize():
            # 1/d for both heads on DVE, table-free: bf16 bit-trick seed +
            # 2 Newton steps in cheap 2x/4x-accelerated bf16 ops.
            db3 = rc_pool.tile([2, 2, 512], BF16, tag="db")
            nc.sync.dma_start(out=db3[0:1, 0, :], in_=stfs[(he, 0)][64:65, :])
            nc.sync.dma_start(out=db3[0:1, 1, :], in_=stfs[(he, 1)][64:65, :])
            nc.sync.dma_start(out=db3[1:2, 0, :], in_=stfs[(ho, 0)][64:65, :])
            nc.sync.dma_start(out=db3[1:2, 1, :], in_=stfs[(ho, 1)][64:65, :])
            db = db3.rearrange("p n t -> p (n t)")
            y0 = rc_pool.tile([2, T], BF16, tag="y0")
            nc.vector.tensor_scalar(
                out=y0.bitcast(I16),
                in0=db.bitcast(I16),
                scalar1=-1.0,
                scalar2=RCP_MAGIC,
                op0=OP.mult,
                op1=OP.add,
            )
            tt = rc_pool.tile([2, T], BF16, tag="tt")
            y1 = rc_pool.tile([2, T], BF16, tag="y1")
            nc.vector.tensor_tensor(out=tt, in0=db, in1=y0, op=OP.mult)
            nc.vector.tensor_scalar(
                out=tt, in0=tt, scalar1=-1.0, scalar2=2.0, op0=OP.mult, op1=OP.add
            )
            nc.vector.tensor_tensor(out=y1, in0=y0, in1=tt, op=OP.mult)
            nc.vector.tensor_tensor(out=tt, in0=db, in1=y1, op=OP.mult)
            nc.vector.tensor_scalar(
                out=tt, in0=tt, scalar1=-1.0, scalar2=2.0, op0=OP.mult, op1=OP.add
            )
            di = rc_pool.tile([2, T], BF16, tag="di")
            nc.vector.tensor_tensor(out=di, in0=y1, in1=tt, op=OP.mult)
            rz_o = rc_pool.tile([1, T], BF16, tag="rzo")
            nc.sync.dma_start(out=rz_o, in_=di[1:2, :])
            for h in (he, ho):
                rb = rc_pool.tile([64, T], BF16, tag="rb")
                nc.gpsimd.partition_broadcast(
                    out_ap=rb, in_ap=(di[0:1, :] if h % 2 == 0 else rz_o), channels=64
                )
                for n in range(2):
                    nsl = slice(512 * n, 512 * (n + 1))
                    if h % 2 == 0:
                        nc.vector.tensor_tensor(
                            out=a_sb[0:64, b, jt, nsl],
                            in0=stfs[(h, n)][0:64, :],
                            in1=rb[:, nsl],
                            op=OP.mult,
                        )
                    else:
                        stg = rc_pool.tile([64, 512], BF16, tag=f"stg{n}")
                        nc.vector.tensor_tensor(
                            out=stg, in0=stfs[(h, n)][0:64, :], in1=rb[:, nsl], op=OP.mult
                        )
                        nc.sync.dma_start(out=a_sb[64:128, b, jt, nsl], in_=stg)

        return normalize

    # ---------------- schedule ----------------
    # upfront: full qkv(b0), V(b1); b0 attention boundaries absorb Q,K(b1)
    # m-tiles; b1 attention boundaries absorb proj(b0); proj(b1) at tail.
    for m in range(8):
        qkv_qk_mtile(0, m)
    for s in range(8):
        qkv_v_stile(0, s)
    for s in range(8):
        qkv_v_stile(1, s)

    def load_xr(b, m):
        xr_t = outp.tile([P, T], F32, tag="xr")
        nc.sync.dma_start(out=xr_t, in_=xv[b, m])
        return xr_t

    pending = None
    b1_qkv_order = [(0, 4), (1, 5), (2, 6), (3,)]
    for hp in range(4):
        nz = attn_headpair(0, hp)
        if pending is not None:
            pending()
        pending = nz
        # filler: Q,K(b1) m-tiles per boundary, earliest-needed first
        for m in b1_qkv_order[hp]:
            qkv_qk_mtile(1, m)

    xr_t = load_xr(0, 0)
    for hp in range(4):
        nz = attn_headpair(1, hp)
        if pending is not None:
            pending()
        pending = nz
        if hp == 0:
            qkv_qk_mtile(1, 7)
        if hp == 3:
            # last head-pair: run its normalize immediately so the tail
            # (proj of batch 1) isn't delayed by the deferral
            pending()
            pending = None
        xr_next = load_xr(0, hp + 1) if hp < 3 else load_xr(1, 0)
        proj_mtile(0, hp, xr_t)
        xr_t = xr_next

    for m in range(4):
        xr_next = load_xr(1, m + 1) if m < 3 else None
        proj_mtile(1, m, xr_t)
        xr_t = xr_next


def build(num_devices=NCORES, debug=False):
    from concourse import bacc

    nc = bacc.Bacc(
        "TRN2", target_bir_lowering=False, debug=debug, num_devices=num_devices
    )
    ap = {}

    def inp(name, shape):
        ap[name] = nc.dram_tensor(name, shape, F32, kind="ExternalInput").ap()

    inp("x", [BL, C, T])
    ap["xbf"] = nc.dram_tensor("xbf", [BL, C, T], BF16, kind="ExternalInput").ap()
    ap["wqkvT"] = nc.dram_tensor("wqkvT", [C, 3 * C], BF16, kind="ExternalInput").ap()
    ap["wprojT"] = nc.dram_tensor("wprojT", [C, C], BF16, kind="ExternalInput").ap()
    inp("bqk", [P, 8])
    inp("bv", [C])
    ap["bvbf"] = nc.dram_tensor("bvbf", [1, C], BF16, kind="ExternalInput").ap()
    inp("bp", [P, 4])
    inp("gscale", [P, 4])
    inp("gbias", [P, 4])
    inp("gsel", [P, 8])
    inp("gexp", [8, P])
    out_ap = nc.dram_tensor("out", [BL, C, T], F32, kind="ExternalOutput").ap()

    with tile.TileContext(nc) as tc:
        with ExitStack() as ctx:
            tc._ctx = ctx
            _kernel_body(nc, tc, ap, out_ap)
    nc.compile()
    return nc


def host_prep(x, gn_scale, gn_bias, w_qkv, b_qkv, w_proj, b_proj):
    """Build the shared (weight) input arrays and the full [16,512,1024] x."""
    xr = np.ascontiguousarray(np.asarray(x, np.float32).reshape(B, C, T))
    w_qkv = np.asarray(w_qkv, np.float32)
    b_qkv = np.asarray(b_qkv, np.float32)
    # permute interleaved [head, (q,k,v), ch] rows -> [(q,k,v), head, ch]
    perm = np.array(
        [h * 3 * CH + w * CH + c for w in range(3) for h in range(NH) for c in range(CH)],
        dtype=np.int64,
    )
    wq_p = w_qkv[perm].copy()
    bq_p = b_qkv[perm].copy()
    wq_p[:C] *= 0.125  # attention scale (1/sqrt(sqrt(ch)))^2 folded into Q
    bq_p[:C] *= 0.125
    import ml_dtypes

    shared = {
        "wqkvT": np.ascontiguousarray(wq_p.T).astype(ml_dtypes.bfloat16),
        "wprojT": np.ascontiguousarray(np.asarray(w_proj, np.float32).T).astype(
            ml_dtypes.bfloat16
        ),
        "bqk": np.ascontiguousarray(bq_p[: 2 * C].reshape(8, P).T),
        "bv": np.ascontiguousarray(bq_p[2 * C :]),
        "bvbf": np.ascontiguousarray(bq_p[2 * C :].reshape(1, C)).astype(
            ml_dtypes.bfloat16
        ),
        "bp": np.ascontiguousarray(np.asarray(b_proj, np.float32).reshape(4, P).T),
        "gscale": np.ascontiguousarray(
            np.asarray(gn_scale, np.float32).reshape(4, P).T
        ),
        "gbias": np.ascontiguousarray(np.asarray(gn_bias, np.float32).reshape(4, P).T),
        "gsel": np.ascontiguousarray(
            (np.arange(P)[:, None] // GS == np.arange(8)[None, :]).astype(np.float32)
        ),
        "gexp": np.ascontiguousarray(
            (np.arange(8)[:, None] == np.arange(P)[None, :] // GS).astype(np.float32)
        ),
    }
    return xr, shared


_NC_CACHE = {}


def kernel(x, gn_scale, gn_bias, w_qkv, b_qkv, w_proj, b_proj):
    xr, shared = host_prep(x, gn_scale, gn_bias, w_qkv, b_qkv, w_proj, b_proj)
    if "nc" not in _NC_CACHE:
        _NC_CACHE["nc"] = build()
    nc = _NC_CACHE["nc"]
    import ml_dtypes

    xbf = xr.astype(ml_dtypes.bfloat16)
    in_maps = [
        {
            "x": np.ascontiguousarray(xr[i * BL : (i + 1) * BL]),
            "xbf": np.ascontiguousarray(xbf[i * BL : (i + 1) * BL]),
            **shared,
        }
        for i in range(NCORES)
    ]
    res = run_bass_kernel_spmd(nc, in_maps, core_ids=list(range(NCORES)))
    out = np.concatenate([res.results[i]["out"] for i in range(NCORES)], axis=0)
    return np.ascontiguousarray(out.reshape(B, C, 32, 32).astype(np.float32))
